# revision 15
# baseline (speedup 1.0000x reference)
"""Trainium2 Bass kernel for a cross-attention transformer layer.

Sharding: data-parallel over query rows. 8 cores; core c handles batch c//2,
query-row half c%2 (1024 rows each), with that batch's full context (2048 rows)
replicated to the 2 cores sharing a batch. No collectives needed.

Per-core kernel layout strategy (all matmuls bf16 with f32 PSUM accumulation):
  - LayerNorm in token-major, then PE-transpose activations to feature-major.
  - Q^T, K^T computed directly in [d, token] layout; V in [token, d] layout.
  - Attention scores as S^T[j, i] = K^T_chunk^T @ Q^T (contraction over head dim).
  - exp on ScalarE straight out of PSUM with the 1/sqrt(dh) scale folded in
    (max-subtraction skipped: logits are O(+-4), exp is safe in f32).
  - AV as [V | ones]^T @ A^T -> O^T plus softmax row-sums in one matmul.
  - Normalization by row-sum applied to O^T via gpsimd partition-broadcast +
    vector multiply; K/V-projection bias enters here (per-partition add).
  - O-projection consumes O^T directly as the stationary operand.
  - MLP with transposed hidden activations; silu+bias fused on ScalarE.
  - LN gains (g) folded into the weight matrices host-side; LN biases (b)
    folded into per-output-feature bias vectors host-side.

SBUF is managed with phase-scoped tile pools on both heap sides (pools are
stack-allocated for their context lifetime, so overlapping phase lifetimes go
on opposite sides).
"""

import numpy as np
import ml_dtypes
import orjson

import concourse.bass as bass
import concourse.tile as tile
from concourse import mybir
from concourse.bass_utils import run_bass_kernel_spmd
from concourse.masks import make_identity


# --------------------------------------------------------------------------
# This container's walrus build caps sync commands at 2 per instruction, but
# Tile freely emits 3+ waits (engine sems + several HW-DMA-queue sems).
# Post-process the BIR before walrus: move excess waits onto EventSemaphore
# carrier instructions inserted just before the over-subscribed instruction
# on the same engine (engines are in-order, so this is semantically
# identical). Updates never exceed the cap in this program.
# --------------------------------------------------------------------------
_MAX_SYNC = 1


def _split_excess_sync(bir: bytes) -> bytes:
    data = orjson.loads(bir)
    n = [0]

    def carrier(inst, waits=()):
        n[0] += 1
        return {
            "debug": inst.get("debug", 0),
            "engine": inst["engine"],
            "ins": [],
            "outs": [],
            "name": f"wsp-{n[0]}-{inst['name']}",
            "opcode": "EventSemaphore",
            "sync_info": {"on_update": [], "on_wait": list(waits)},
        }

    def fix_block(blk):
        insts = blk.get("instructions")
        if insts:
            out = []
            changed = False
            for inst in insts:
                si = inst.get("sync_info")
                waits = (si or {}).get("on_wait") or []
                # HWDGE DMA pseudo-instructions only have room for ONE wait
                cap = _MAX_SYNC
                if len(waits) > cap:
                    keep, excess = waits[:cap], waits[cap:]
                    si["on_wait"] = keep
                    for i in range(0, len(excess), _MAX_SYNC):
                        out.append(carrier(inst, waits=excess[i:i + _MAX_SYNC]))
                    changed = True
                out.append(inst)
            if changed:
                blk["instructions"] = out
        for sub in blk.get("blocks", []) or []:
            fix_block(sub)

    for fn in data.get("functions", []):
        for blk in fn.get("blocks", []):
            fix_block(blk)
    return orjson.dumps(data)


_SPLITTER_INSTALLED = [False]


def _install_wait_splitter():
    if _SPLITTER_INSTALLED[0]:
        return
    _SPLITTER_INSTALLED[0] = True
    import concourse.bass_utils as bu
    import concourse.bass2jax as b2j

    orig = bu.compile_bir_kernel

    def patched(bir_json, tmpdir, neff_name="file.neff"):
        return orig(_split_excess_sync(bir_json), tmpdir, neff_name)

    bu.compile_bir_kernel = patched
    b2j.compile_bir_kernel = patched

BF16 = mybir.dt.bfloat16
F32 = mybir.dt.float32
AF = mybir.ActivationFunctionType
ALU = mybir.AluOpType

P = 128
D = 1024           # model dim
H = 16             # heads
DH = 64            # head dim
MLP = 4096
EPS = 1e-5
SCALE = DH ** -0.5

CC = D // P        # 8 feature chunks
MC = MLP // P      # 32 mlp-hidden chunks


def _bcast_ap(ap, nparts):
    """Broadcast an AP across nparts partitions (prepend a 0-step dim)."""
    return bass.AP(tensor=ap.tensor, offset=ap.offset, ap=[[0, nparts]] + list(ap.ap))


def build_nc(NQ=1024, NC=2048, sim_safe=False):
    """Build the single-core Bass program (same program runs SPMD on all cores)."""
    IC = NQ // P           # query-row chunks
    JC = NC // P           # context-row chunks
    IS = min(512, NQ)      # i-slice width
    NIS = NQ // IS
    JS = min(512, NC)      # j-slice width (for K^T copy granularity)
    NJS = NC // JS
    ES = 512               # output-feature slice width
    NES = D // ES

    nc = bass.Bass()

    # ---- DRAM tensors (per-core inputs) ----
    x_h = nc.dram_tensor("x", [NQ, D], F32, kind="ExternalInput")
    ctx_h = nc.dram_tensor("ctx", [NC, D], F32, kind="ExternalInput")
    wq_h = nc.dram_tensor("wq", [CC, P, D], BF16, kind="ExternalInput")
    wk_h = nc.dram_tensor("wk", [CC, P, D], BF16, kind="ExternalInput")
    wv_h = nc.dram_tensor("wv", [CC, P, D], BF16, kind="ExternalInput")
    wo_h = nc.dram_tensor("wo", [CC, P, D], BF16, kind="ExternalInput")
    w1_h = nc.dram_tensor("w1", [MC, P, CC, P], BF16, kind="ExternalInput")
    w2_h = nc.dram_tensor("w2", [MC, P, D], BF16, kind="ExternalInput")
    bq_h = nc.dram_tensor("bq", [D], F32, kind="ExternalInput")
    bk_h = nc.dram_tensor("bk", [D], F32, kind="ExternalInput")
    bvv_h = nc.dram_tensor("bvv", [D], F32, kind="ExternalInput")
    b1_h = nc.dram_tensor("b1f", [MLP], F32, kind="ExternalInput")
    bo_h = nc.dram_tensor("bo", [D], F32, kind="ExternalInput")
    b2_h = nc.dram_tensor("b2", [D], F32, kind="ExternalInput")
    out_h = nc.dram_tensor("out", [NQ, D], F32, kind="ExternalOutput")

    x_r = x_h[:].rearrange("(ic p) d -> ic p d", p=P)
    ctx_r = ctx_h[:].rearrange("(jc p) d -> jc p d", p=P)
    out_r = out_h[:].rearrange("(ic p) d -> ic p d", p=P)

    with tile.TileContext(nc) as tc:
        with (
            tc.tile_pool(name="consts", bufs=1) as consts,
            tc.tile_pool(name="xio", bufs=3) as xpool,
            tc.tile_pool(name="stats", bufs=4) as spool,
            tc.tile_pool(name="psum", bufs=6, space="PSUM") as psum,
            tc.tile_pool(name="psumt", bufs=2, space="PSUM") as psumt,
        ):
            # ---- constants ----
            ident = consts.tile([P, P], BF16)
            make_identity(nc, ident)
            ones1 = consts.tile([1, DH], F32)
            nc.vector.memset(ones1, 1.0)
            eps_t = consts.tile([P, 1], F32)
            nc.vector.memset(eps_t, EPS)
            bq_sb = consts.tile([P, CC], F32)
            nc.sync.dma_start(out=bq_sb, in_=bq_h[:].rearrange("(c p) -> p c", p=P))
            bk_sb = consts.tile([P, CC], F32)
            nc.sync.dma_start(out=bk_sb, in_=bk_h[:].rearrange("(c p) -> p c", p=P))
            bvv_sb = consts.tile([DH, H], F32)
            nc.sync.dma_start(out=bvv_sb, in_=bvv_h[:].rearrange("(h p) -> p h", p=DH))
            b1_sb = consts.tile([P, MC], F32)
            nc.sync.dma_start(out=b1_sb, in_=b1_h[:].rearrange("(c p) -> p c", p=P))
            bo_bc = consts.tile([P, D], F32)
            nc.sync.dma_start(out=bo_bc, in_=_bcast_ap(bo_h[:], P))
            b2_bc = consts.tile([P, D], F32)
            nc.sync.dma_start(out=b2_bc, in_=_bcast_ap(b2_h[:], P))

            def layer_norm(xt):
                """xt: [P, D] f32 sbuf tile -> [P, D] bf16 normalized (no g/b)."""
                stats = spool.tile([P, 2, 6], F32, tag="stats")
                nc.vector.bn_stats(out=stats[:, 0, :], in_=xt[:, 0:512])
                nc.vector.bn_stats(out=stats[:, 1, :], in_=xt[:, 512:1024])
                mv = spool.tile([P, 2], F32, tag="mv")
                nc.vector.bn_aggr(out=mv, in_=stats)
                std = spool.tile([P, 1], F32, tag="std")
                nc.scalar.activation(out=std, in_=mv[:, 1:2], func=AF.Sqrt, bias=eps_t)
                rst = spool.tile([P, 1], F32, tag="rst")
                nc.vector.reciprocal(out=rst, in_=std)
                xh = spool.tile([P, D], BF16, tag="lnout")
                nc.vector.tensor_scalar(
                    out=xh, in0=xt, scalar1=mv[:, 0:1], scalar2=rst,
                    op0=ALU.subtract, op1=ALU.mult,
                )
                return xh

            def transpose_into(dst, xh, t):
                """xh [P, D] bf16 -> dst[:, cc, t*P:(t+1)*P] for each cc."""
                for cc in range(CC):
                    ps = psumt.tile([P, P], BF16, tag="pst")
                    nc.tensor.transpose(ps, xh[:, cc * P:(cc + 1) * P], ident)
                    nc.vector.tensor_copy(out=dst[:, cc, t * P:(t + 1) * P], in_=ps)

            with tc.tile_pool(name="ln_t", bufs=1) as lnp:       # left
                xhT = lnp.tile([P, CC, NQ], BF16, tag="xhT")
                chT = lnp.tile([P, CC, NC], BF16, tag="chT")

                # ---- Phase 1: LayerNorms + transposes ----
                for t in range(IC):
                    xt = xpool.tile([P, D], F32, tag="xio")
                    nc.sync.dma_start(out=xt, in_=x_r[t])
                    transpose_into(xhT, layer_norm(xt), t)
                for t in range(JC):
                    ct = xpool.tile([P, D], F32, tag="xio")
                    nc.sync.dma_start(out=ct, in_=ctx_r[t])
                    transpose_into(chT, layer_norm(ct), t)

                qkv = tc.alloc_tile_pool(name="qkv", bufs=1, side="right")
                QT = qkv.tile([P, CC, NQ], BF16, tag="QT")
                KT = qkv.tile([P, CC, NC], BF16, tag="KT")
                Vp = qkv.tile([P, JC, H, DH + 1], BF16, tag="Vp")

                # ---- Phase 2a: Q^T = Wq'^T @ xh^T ----
                with tc.tile_pool(name="wq", bufs=1, side="right") as wp:
                    wq_sb = wp.tile([P, CC, D], BF16, tag="w")
                    nc.sync.dma_start(out=wq_sb, in_=wq_h[:].rearrange("c p m -> p c m"))
                    for mc in range(CC):
                        for i in range(NIS):
                            ps = psum.tile([P, 512], F32, tag="ps")
                            for cc in range(CC):
                                nc.tensor.matmul(
                                    ps[:, 0:IS],
                                    lhsT=wq_sb[:, cc, mc * P:(mc + 1) * P],
                                    rhs=xhT[:, cc, i * IS:(i + 1) * IS],
                                    start=(cc == 0), stop=(cc == CC - 1),
                                )
                            nc.vector.tensor_scalar_add(
                                out=QT[:, mc, i * IS:(i + 1) * IS], in0=ps[:, 0:IS],
                                scalar1=bq_sb[:, mc:mc + 1],
                            )

                # ---- Phase 2b: K^T = Wk'^T @ ch^T ----
                with tc.tile_pool(name="wk", bufs=1, side="right") as wp:
                    wk_sb = wp.tile([P, CC, D], BF16, tag="w")
                    nc.sync.dma_start(out=wk_sb, in_=wk_h[:].rearrange("c p m -> p c m"))
                    for mc in range(CC):
                        for j in range(NJS):
                            ps = psum.tile([P, 512], F32, tag="ps")
                            for cc in range(CC):
                                nc.tensor.matmul(
                                    ps[:, 0:JS],
                                    lhsT=wk_sb[:, cc, mc * P:(mc + 1) * P],
                                    rhs=chT[:, cc, j * JS:(j + 1) * JS],
                                    start=(cc == 0), stop=(cc == CC - 1),
                                )
                            nc.vector.tensor_scalar_add(
                                out=KT[:, mc, j * JS:(j + 1) * JS], in0=ps[:, 0:JS],
                                scalar1=bk_sb[:, mc:mc + 1],
                            )

                # ---- Phase 2c: V = ch @ Wv' (token-major) ----
                with tc.tile_pool(name="wv", bufs=1, side="right") as wp:
                    wv_sb = wp.tile([P, CC, D], BF16, tag="w")
                    nc.sync.dma_start(out=wv_sb, in_=wv_h[:].rearrange("c p m -> p c m"))
                    for jc in range(JC):
                        for ds in range(2):
                            ps = psum.tile([P, 512], F32, tag="ps")
                            for cc in range(CC):
                                nc.tensor.matmul(
                                    ps,
                                    lhsT=chT[:, cc, jc * P:(jc + 1) * P],
                                    rhs=wv_sb[:, cc, ds * 512:(ds + 1) * 512],
                                    start=(cc == 0), stop=(cc == CC - 1),
                                )
                            nc.vector.tensor_copy(
                                out=Vp[:, jc, ds * 8:(ds + 1) * 8, 0:DH],
                                in_=ps.rearrange("p (h d) -> p h d", d=DH),
                            )
                    nc.vector.memset(Vp[:, :, :, DH:DH + 1], 1.0)
            # ln_t (xhT/chT) released here

            with tc.tile_pool(name="ot", bufs=1) as otp:          # left
                OT = otp.tile([P, CC, NQ], BF16, tag="OT")

                # ---- Phase 3: attention ----
                with tc.tile_pool(name="attn", bufs=2) as atn:    # left
                    for hp in range(H // 2):
                        for i in range(NIS):
                            for s in (0, 1):
                                h = 2 * hp + s
                                b0 = DH * s
                                av = psum.tile([P, 512], F32, tag="ps")
                                for jc in range(JC):
                                    sp = psum.tile([P, 512], F32, tag="ps")
                                    nc.tensor.matmul(
                                        sp[:, 0:IS],
                                        lhsT=KT[b0:b0 + DH, hp, jc * P:(jc + 1) * P],
                                        rhs=QT[b0:b0 + DH, hp, i * IS:(i + 1) * IS],
                                        start=True, stop=True,
                                    )
                                    at = atn.tile([P, IS], BF16, tag="at")
                                    nc.scalar.activation(out=at, in_=sp[:, 0:IS],
                                                         func=AF.Exp, scale=SCALE)
                                    nc.tensor.matmul(
                                        av[0:DH + 1, 0:IS],
                                        lhsT=Vp[:, jc, h, :],
                                        rhs=at,
                                        start=(jc == 0), stop=(jc == JC - 1),
                                    )
                                # normalize by softmax row-sum (row DH of av):
                                # broadcast 1/r across partitions via K=1
                                # ones-matmul on the PE, then multiply.
                                rrec = atn.tile([1, IS], F32, tag="rrec")
                                nc.vector.reciprocal(out=rrec, in_=av[DH:DH + 1, 0:IS])
                                bc_ps = psum.tile([P, 512], F32, tag="ps")
                                nc.tensor.matmul(bc_ps[0:DH, 0:IS], lhsT=ones1,
                                                 rhs=rrec, start=True, stop=True)
                                rbc = atn.tile([DH, IS], F32, tag="rbc")
                                nc.vector.tensor_copy(out=rbc, in_=bc_ps[0:DH, 0:IS])
                                otf = atn.tile([DH, IS], F32, tag="otf")
                                nc.vector.tensor_mul(otf, av[0:DH, 0:IS], rbc)
                                if s == 0:
                                    nc.scalar.activation(
                                        out=OT[0:DH, hp, i * IS:(i + 1) * IS], in_=otf,
                                        func=AF.Identity, bias=bvv_sb[:, h:h + 1],
                                    )
                                else:
                                    otb = atn.tile([DH, IS], BF16, tag="otb")
                                    nc.scalar.activation(out=otb, in_=otf,
                                                         func=AF.Identity,
                                                         bias=bvv_sb[:, h:h + 1])
                                    # partition shift 0:64 -> 64:128 via DMA
                                    nc.sync.dma_start(
                                        out=OT[DH:P, hp, i * IS:(i + 1) * IS], in_=otb)
                qkv.release()  # QT/KT/Vp dead (right side)

                yp = tc.alloc_tile_pool(name="yt", bufs=1, side="right")
                yt = yp.tile([P, IC, D], F32, tag="yt")

                # ---- Phase 4: O-projection + residual ----
                with tc.tile_pool(name="wo", bufs=1, side="right") as wp:
                    wo_sb = wp.tile([P, CC, D], BF16, tag="w")
                    nc.sync.dma_start(out=wo_sb, in_=wo_h[:].rearrange("c p m -> p c m"))
                    for ic in range(IC):
                        xr = xpool.tile([P, D], F32, tag="xio")
                        nc.sync.dma_start(out=xr, in_=x_r[ic])
                        for e in range(NES):
                            ps = psum.tile([P, 512], F32, tag="ps")
                            for dc in range(CC):
                                nc.tensor.matmul(
                                    ps,
                                    lhsT=OT[:, dc, ic * P:(ic + 1) * P],
                                    rhs=wo_sb[:, dc, e * ES:(e + 1) * ES],
                                    start=(dc == 0), stop=(dc == CC - 1),
                                )
                            nc.vector.scalar_tensor_tensor(
                                out=yt[:, ic, e * ES:(e + 1) * ES], in0=ps, scalar=0.0,
                                in1=xr[:, e * ES:(e + 1) * ES],
                                op0=ALU.add, op1=ALU.add,
                            )
                            nc.vector.tensor_add(
                                out=yt[:, ic, e * ES:(e + 1) * ES],
                                in0=yt[:, ic, e * ES:(e + 1) * ES],
                                in1=bo_bc[:, e * ES:(e + 1) * ES],
                            )
            # OT released (left)

            with tc.tile_pool(name="mlp_t", bufs=1) as mtp:       # left
                yhT = mtp.tile([P, CC, NQ], BF16, tag="yhT")
                s1T = mtp.tile([P, MC, NQ], BF16, tag="s1T")

                # ---- Phase 5a: LN(y) + transpose ----
                for t in range(IC):
                    transpose_into(yhT, layer_norm(yt[:, t, :]), t)

                # ---- Phase 5b: hidden = silu(W1'^T @ yh^T + b1') ----
                with tc.tile_pool(name="w1s", bufs=3) as wp:
                    for mc in range(MC):
                        w1t = wp.tile([P, CC, P], BF16, tag="w")
                        nc.sync.dma_start(out=w1t, in_=w1_h[mc])
                        for i in range(NIS):
                            ps = psum.tile([P, 512], F32, tag="ps")
                            for cc in range(CC):
                                nc.tensor.matmul(
                                    ps[:, 0:IS],
                                    lhsT=w1t[:, cc, :],
                                    rhs=yhT[:, cc, i * IS:(i + 1) * IS],
                                    start=(cc == 0), stop=(cc == CC - 1),
                                )
                            if sim_safe:
                                # CoreSim lacks Silu: z*sigmoid(z) decomposition
                                zb = spool.tile([P, IS], F32, tag="zb")
                                nc.vector.tensor_scalar_add(
                                    out=zb, in0=ps[:, 0:IS],
                                    scalar1=b1_sb[:, mc:mc + 1])
                                sg = spool.tile([P, IS], F32, tag="sg")
                                nc.scalar.activation(out=sg, in_=zb, func=AF.Sigmoid)
                                nc.vector.tensor_mul(
                                    s1T[:, mc, i * IS:(i + 1) * IS], zb, sg)
                            else:
                                nc.scalar.activation(
                                    out=s1T[:, mc, i * IS:(i + 1) * IS],
                                    in_=ps[:, 0:IS],
                                    func=AF.Silu, bias=b1_sb[:, mc:mc + 1],
                                )

                # ---- Phase 5c: out = y + s1 @ W2 + b2 ----
                with tc.tile_pool(name="w2s", bufs=1) as wp:
                    for e in range(NES):
                        w2t = wp.tile([P, MC, ES], BF16, tag="w")
                        nc.sync.dma_start(
                            out=w2t,
                            in_=w2_h[:, :, e * ES:(e + 1) * ES].rearrange(
                                "c p m -> p c m"))
                        for ic in range(IC):
                            ps = psum.tile([P, 512], F32, tag="ps")
                            for mc in range(MC):
                                nc.tensor.matmul(
                                    ps,
                                    lhsT=s1T[:, mc, ic * P:(ic + 1) * P],
                                    rhs=w2t[:, mc, :],
                                    start=(mc == 0), stop=(mc == MC - 1),
                                )
                            of = spool.tile([P, ES], F32, tag="of")
                            nc.vector.scalar_tensor_tensor(
                                out=of, in0=ps, scalar=0.0,
                                in1=yt[:, ic, e * ES:(e + 1) * ES],
                                op0=ALU.add, op1=ALU.add,
                            )
                            nc.vector.tensor_add(
                                out=of, in0=of, in1=b2_bc[:, e * ES:(e + 1) * ES])
                            nc.sync.dma_start(
                                out=out_r[ic, :, e * ES:(e + 1) * ES], in_=of)
                yp.release()

    return nc


def prepare_weights(Wq, Wk, Wv, Wo, bo, g_q, b_q, g_k, b_k, g_m, b_m, W1, b1, W2, b2):
    """Host-side weight packing shared by all cores."""
    bf = ml_dtypes.bfloat16

    def chunk_rows(w):  # [D, M] -> [CC, P, M]
        return np.ascontiguousarray(w.reshape(CC, P, w.shape[1]).astype(bf))

    wq_p = chunk_rows(g_q[:, None] * Wq)
    wk_p = chunk_rows(g_k[:, None] * Wk)
    wv_p = chunk_rows(g_k[:, None] * Wv)
    wo_p = chunk_rows(Wo)
    w1g = g_m[:, None] * W1                      # [D, MLP]
    w1_p = np.ascontiguousarray(
        w1g.reshape(CC, P, MC, P).transpose(2, 1, 0, 3).astype(bf))  # [MC,P,CC,P]
    w2_p = np.ascontiguousarray(W2.reshape(MC, P, D).astype(bf))

    f32 = np.float32
    return {
        "wq": wq_p, "wk": wk_p, "wv": wv_p, "wo": wo_p, "w1": w1_p, "w2": w2_p,
        "bq": (b_q @ Wq).astype(f32),
        "bk": (b_k @ Wk).astype(f32),
        "bvv": (b_k @ Wv).astype(f32),
        "b1f": (b_m @ W1 + b1).astype(f32),
        "bo": bo.astype(f32),
        "b2": b2.astype(f32),
    }


_NC_CACHE = {}


def _get_nc(NQ=1024, NC=2048):
    key = (NQ, NC)
    if key not in _NC_CACHE:
        _NC_CACHE[key] = build_nc(NQ, NC)
    return _NC_CACHE[key]


def make_in_maps(inputs):
    """Shard FULL inputs into 8 per-core input maps."""
    x = np.asarray(inputs["x"], np.float32)
    context = np.asarray(inputs["context"], np.float32)
    B, N, _ = x.shape
    half = N // 2
    w = prepare_weights(
        np.asarray(inputs["Wq"], np.float32), np.asarray(inputs["Wk"], np.float32),
        np.asarray(inputs["Wv"], np.float32), np.asarray(inputs["Wo"], np.float32),
        np.asarray(inputs["bo"], np.float32),
        np.asarray(inputs["g_q"], np.float32), np.asarray(inputs["b_q"], np.float32),
        np.asarray(inputs["g_k"], np.float32), np.asarray(inputs["b_k"], np.float32),
        np.asarray(inputs["g_m"], np.float32), np.asarray(inputs["b_m"], np.float32),
        np.asarray(inputs["W1"], np.float32), np.asarray(inputs["b1"], np.float32),
        np.asarray(inputs["W2"], np.float32), np.asarray(inputs["b2"], np.float32),
    )
    in_maps = []
    for c in range(8):
        b, hf = c // 2, c % 2
        m = dict(w)
        m["x"] = np.ascontiguousarray(x[b, hf * half:(hf + 1) * half])
        m["ctx"] = np.ascontiguousarray(context[b])
        in_maps.append(m)
    return in_maps


def run_cores(inputs, trace=False):
    """Shard inputs, run the SPMD kernel on 8 cores, gather the output."""
    _install_wait_splitter()
    x = np.asarray(inputs["x"], np.float32)
    context = np.asarray(inputs["context"], np.float32)
    B, N, _ = x.shape
    half = N // 2
    in_maps = make_in_maps(inputs)

    nc = _get_nc(half, context.shape[1])
    res = run_bass_kernel_spmd(nc, in_maps, list(range(8)), trace=trace)

    out = np.empty((B, N, D), np.float32)
    for c in range(8):
        b, hf = c // 2, c % 2
        out[b, hf * half:(hf + 1) * half] = res.results[c]["out"]
    return out, res


def kernel(**inputs):
    out, _ = run_cores(inputs, trace=False)
    return out
